# revision 1
# baseline (speedup 1.0000x reference)
"""MoE regressor (E=16, H=1024, B=4096, top-2) on 8 trn2 NeuronCores.

Expert-parallel sharding: each core owns 2 experts. The host computes the
top-2 routing (replicated router, fp32) and dispatches each expert's
tokens to its core (all-to-all style gather done during input sharding);
each core runs the two-expert 2-layer MLP (f32r matmuls on the PE array)
over its gathered token set and returns per-slot expert outputs; the
host applies the softmax combine weights during unsharding (scatter-add)
and sums the per-core partials.

Self-contained: hardcodes all shapes.
"""

import numpy as np

import concourse.bass as bass  # noqa: F401
from concourse import bacc
import concourse.mybir as mybir
import concourse.tile as tile
from concourse.bass_utils import run_bass_kernel_spmd
from concourse.masks import make_identity

P = 128
B = 4096
H = 1024
E = 16
NCORES = 8
EPC = E // NCORES  # experts per core = 2

F32 = mybir.dt.float32
F32R = mybir.dt.float32r

_CACHE = {}


def _build(C):
    """Per-core kernel: dense 2-layer MLP over C gathered tokens x 2 experts."""
    CN = C // P
    nc = bacc.Bacc(None, target_bir_lowering=False)

    # ge: gathered token embeddings per expert, [e, p, c2, h] slot r = c2*128+p
    ge = nc.dram_tensor("ge", (EPC, P, CN, H), F32, kind="ExternalInput")
    w1s = nc.dram_tensor("w1s", (EPC, 8, P, 8, P), F32, kind="ExternalInput")
    b1s = nc.dram_tensor("b1s", (P, 8, EPC), F32, kind="ExternalInput")
    w2s = nc.dram_tensor("w2s", (P, 8, EPC), F32, kind="ExternalInput")
    b2s = nc.dram_tensor("b2s", (1, EPC), F32, kind="ExternalInput")
    out2 = nc.dram_tensor("out2", (EPC, C), F32, kind="ExternalOutput")

    with tile.TileContext(nc) as tc:
        with (
            tc.tile_pool(name="const", bufs=1) as cpool,
            tc.tile_pool(name="sb", bufs=2) as sb,
            tc.tile_pool(name="wpool", bufs=3) as wpool,
            tc.tile_pool(name="pst", bufs=2, space="PSUM") as pst_pool,
            tc.tile_pool(name="ps1", bufs=2, space="PSUM") as ps1_pool,
            tc.tile_pool(name="ps2", bufs=1, space="PSUM") as ps2_pool,
        ):
            ident = cpool.tile([P, P], F32)
            make_identity(nc, ident)
            w2_sb = cpool.tile([P, 8, EPC], F32R)
            nc.sync.dma_start(w2_sb, w2s[:, :, :].bitcast(F32R))
            b1_sb = cpool.tile([P, 8, EPC], F32)
            nc.sync.dma_start(b1_sb, b1s[:, :, :])
            b2_sb = cpool.tile([1, EPC], F32)
            nc.sync.dma_start(b2_sb, b2s[:, :])

            for e in range(EPC):
                # ---- load this expert's gathered tokens ----------------
                gath = sb.tile([P, CN, H], F32, tag="gath")
                nc.sync.dma_start(gath, ge[e])
                # ---- transpose to [H, C] -------------------------------
                embTg = sb.tile([P, 8, C], F32R, tag="embTg")
                for hb in range(8):
                    for c0 in range(0, CN, 4):
                        cw = min(4, CN - c0)
                        pt = pst_pool.tile([P, 512], F32, tag="pt")
                        for j in range(cw):
                            nc.tensor.transpose(
                                pt[:, j * P:(j + 1) * P],
                                gath[:, c0 + j, hb * P:(hb + 1) * P],
                                ident,
                            )
                        nc.vector.tensor_copy(
                            out=embTg[:, hb, c0 * P:(c0 + cw) * P],
                            in_=pt[:, :cw * P],
                        )
                # ---- 2-layer MLP ---------------------------------------
                h_sb = sb.tile([P, 8, C], F32R, tag="h")
                p2a = ps2_pool.tile([1, 512], F32, tag="p2a")
                p2b = ps2_pool.tile([1, C - 512], F32, tag="p2b")
                for m in range(8):
                    w1m = wpool.tile([P, 8, P], F32R, tag="w1m")
                    nc.sync.dma_start(w1m, w1s[e, m].bitcast(F32R))
                    p1a = ps1_pool.tile([P, 512], F32, tag="p1a")
                    p1b = ps1_pool.tile([P, C - 512], F32, tag="p1b")
                    for k in range(8):
                        nc.tensor.matmul(
                            p1a, w1m[:, k], embTg[:, k, :512],
                            start=(k == 0), stop=(k == 7),
                        )
                    for k in range(8):
                        nc.tensor.matmul(
                            p1b, w1m[:, k], embTg[:, k, 512:],
                            start=(k == 0), stop=(k == 7),
                        )
                    nc.scalar.activation(
                        h_sb[:, m, :512], p1a,
                        mybir.ActivationFunctionType.Relu,
                        bias=b1_sb[:, m, e:e + 1],
                    )
                    nc.scalar.activation(
                        h_sb[:, m, 512:], p1b,
                        mybir.ActivationFunctionType.Relu,
                        bias=b1_sb[:, m, e:e + 1],
                    )
                    nc.tensor.matmul(
                        p2a, w2_sb[:, m, e:e + 1], h_sb[:, m, :512],
                        start=(m == 0), stop=(m == 7),
                    )
                    nc.tensor.matmul(
                        p2b, w2_sb[:, m, e:e + 1], h_sb[:, m, 512:],
                        start=(m == 0), stop=(m == 7),
                    )
                out2_sb = sb.tile([1, C], F32, tag="out2")
                nc.vector.tensor_scalar_add(out2_sb[:, :512], p2a, b2_sb[:, e:e + 1])
                nc.vector.tensor_scalar_add(out2_sb[:, 512:], p2b, b2_sb[:, e:e + 1])
                nc.sync.dma_start(out2[e, :][None, :], out2_sb)
    nc.finalize()
    return nc


def _route_host(emb, rw, rb):
    logits = emb.astype(np.float32) @ rw.astype(np.float32) + rb.astype(np.float32)
    i1 = np.argmax(logits, axis=1)
    l1 = logits[np.arange(B), i1]
    l2m = logits.copy()
    l2m[np.arange(B), i1] = -np.inf
    i2 = np.argmax(l2m, axis=1)
    l2 = l2m[np.arange(B), i2]
    d = np.exp(l2 - l1)
    w1 = (1.0 / (1.0 + d)).astype(np.float32)
    w2 = (1.0 - w1).astype(np.float32)
    comb = np.zeros((B, E), np.float32)
    comb[np.arange(B), i1] = w1
    comb[np.arange(B), i2] = w2
    return comb


def kernel(embeddings, router_w, router_b, w1, b1, w2, b2):
    emb = np.ascontiguousarray(np.asarray(embeddings, dtype=np.float32))
    rw = np.asarray(router_w, np.float32)
    rb = np.asarray(router_b, np.float32)
    w1 = np.asarray(w1, np.float32)
    b1 = np.asarray(b1, np.float32)
    w2 = np.asarray(w2, np.float32)
    b2 = np.asarray(b2, np.float32)

    comb = _route_host(emb, rw, rb)
    counts = (comb > 0).sum(axis=0)
    C = 640
    maxc = int(counts.max())
    if maxc > C:
        C = ((maxc + P - 1) // P) * P
    CN = C // P

    if C not in _CACHE:
        _CACHE[C] = _build(C)
    nc = _CACHE[C]

    in_maps = []
    toks = []
    for c in range(NCORES):
        es = [EPC * c + j for j in range(EPC)]
        ge = np.zeros((EPC, P, CN, H), np.float32)
        ctoks = []
        for j, e in enumerate(es):
            ids = np.nonzero(comb[:, e] > 0)[0]
            ctoks.append(ids)
            g = np.zeros((C, H), np.float32)
            g[:len(ids)] = emb[ids]
            # slot r = c2*128 + p  ->  [p, c2, h]
            ge[j] = g.reshape(CN, P, H).transpose(1, 0, 2)
        toks.append(ctoks)
        w1c = np.ascontiguousarray(
            w1[es].reshape(EPC, 8, P, 8, P).transpose(0, 3, 2, 1, 4)
        )
        b1c = np.ascontiguousarray(b1[es].reshape(EPC, 8, P).transpose(2, 1, 0))
        w2c = np.ascontiguousarray(w2[es, :, 0].reshape(EPC, 8, P).transpose(2, 1, 0))
        b2c = np.ascontiguousarray(b2[es, 0].reshape(1, EPC))
        in_maps.append({
            "ge": np.ascontiguousarray(ge),
            "w1s": w1c,
            "b1s": b1c,
            "w2s": w2c,
            "b2s": b2c,
        })

    res = run_bass_kernel_spmd(nc, in_maps, core_ids=list(range(NCORES)))

    out = np.zeros((B,), np.float32)
    for c in range(NCORES):
        o2 = res.results[c]["out2"]  # [EPC, C]
        for j, e in enumerate([EPC * c + jj for jj in range(EPC)]):
            ids = toks[c][j]
            out[ids] += comb[ids, e] * o2[j, :len(ids)]
    return out.reshape(B, 1)



# revision 5
# speedup vs baseline: 1.6230x; 1.6230x over previous
"""MoE regressor (E=16, H=1024, B=4096, top-2) on 8 trn2 NeuronCores.

Expert-parallel, count-aware schedule: the host computes top-2 routing
(replicated router, fp32), sorts experts by token count, and assigns the
8 largest as phase-0 (one per core) and the 8 smallest as phase-1. Phase
lengths C1/C2 are the max count within each phase group, so per-core
slot count is c(1)+c(9) instead of 2*c(1). Token embeddings are gathered,
transposed to [H, C] and cast to bf16 on the host; weights are cast to
bf16 and laid out as matmul lhsT blocks. Each core runs the 2-layer MLP
(bf16 matmuls, fp32 PSUM accumulate) for its two experts; the host
applies the softmax combine weights (scatter-add) on the returned
per-slot outputs.

Self-contained: hardcodes all shapes.
"""

import numpy as np
import ml_dtypes

import concourse.bass as bass  # noqa: F401
from concourse import bacc
import concourse.mybir as mybir
import concourse.tile as tile
from concourse.bass_utils import run_bass_kernel_spmd

P = 128
B = 4096
H = 1024
E = 16
NCORES = 8
NPH = 2  # phases (experts) per core

F32 = mybir.dt.float32
BF16 = mybir.dt.bfloat16
BF_NP = ml_dtypes.bfloat16

_CACHE = {}


def _chunks(C):
    """Split C columns into PSUM-bank-sized chunks (<=512 fp32)."""
    out = []
    c0 = 0
    while c0 < C:
        out.append((c0, min(512, C - c0)))
        c0 += 512
    return out


def _build(C1, C2):
    """Per-core kernel: two experts (phase sizes C1 >= C2), 2-layer MLP."""
    nc = bacc.Bacc(None, target_bir_lowering=False)

    ge0 = nc.dram_tensor("ge0", (P, 8, C1), BF16, kind="ExternalInput")
    ge1 = nc.dram_tensor("ge1", (P, 8, C2), BF16, kind="ExternalInput")
    w1s = nc.dram_tensor("w1s", (NPH, 8, P, 8, P), BF16, kind="ExternalInput")
    b1s = nc.dram_tensor("b1s", (P, 8, NPH), F32, kind="ExternalInput")
    w2s = nc.dram_tensor("w2s", (P, 8, NPH), BF16, kind="ExternalInput")
    b2s = nc.dram_tensor("b2s", (1, NPH), F32, kind="ExternalInput")
    out0 = nc.dram_tensor("out0", (1, C1), F32, kind="ExternalOutput")
    out1 = nc.dram_tensor("out1", (1, C2), F32, kind="ExternalOutput")

    ges = [ge0, ge1]
    outs = [out0, out1]
    Cs = [C1, C2]
    # uniform psum chunk shapes across phases (phase 1 slices into them)
    ch1 = _chunks(C1)

    with tile.TileContext(nc) as tc:
        with (
            tc.tile_pool(name="const", bufs=1) as cpool,
            tc.tile_pool(name="ps1", bufs=2, space="PSUM") as ps1_pool,
            tc.tile_pool(name="ps2", bufs=1, space="PSUM") as ps2_pool,
        ):
            # ---- stage all inputs into SBUF (everything fits) ----------
            b1_sb = cpool.tile([P, 8, NPH], F32)
            w2_sb = cpool.tile([P, 8, NPH], BF16)
            b2_sb = cpool.tile([1, NPH], F32)
            emb_sb = [
                cpool.tile([P, 8, Cs[ph]], BF16, name=f"emb{ph}", tag=f"emb{ph}")
                for ph in range(NPH)
            ]
            w1_sb = cpool.tile([P, NPH, 8, 8, P], BF16)
            h_sb = [
                cpool.tile([P, 8, Cs[ph]], BF16, name=f"h{ph}", tag=f"h{ph}")
                for ph in range(NPH)
            ]
            o_sb = [
                cpool.tile([1, Cs[ph]], F32, name=f"o{ph}", tag=f"o{ph}")
                for ph in range(NPH)
            ]

            # DMA issue order = need order: phase-0 weights m=0 first, then
            # phase-0 embeddings (split for earlier first-matmul), etc.
            nc.sync.dma_start(w1_sb[:, 0, 0], w1s[0, 0])
            nc.sync.dma_start(emb_sb[0][:, :4], ge0[:, :4])
            nc.sync.dma_start(emb_sb[0][:, 4:], ge0[:, 4:])
            nc.sync.dma_start(b1_sb, b1s[:, :, :])
            nc.sync.dma_start(w2_sb, w2s[:, :, :])
            nc.sync.dma_start(b2_sb, b2s[:, :])
            for m in range(1, 8):
                nc.sync.dma_start(w1_sb[:, 0, m], w1s[0, m])
            nc.sync.dma_start(emb_sb[1][:, :4], ge1[:, :4])
            nc.sync.dma_start(emb_sb[1][:, 4:], ge1[:, 4:])
            for m in range(8):
                nc.sync.dma_start(w1_sb[:, 1, m], w1s[1, m])

            for ph in range(NPH):
                C = Cs[ph]
                embT = emb_sb[ph]
                h = h_sb[ph]
                chunks = _chunks(C)
                p2 = [
                    ps2_pool.tile([1, cw1], F32, name=f"p2_{i}", tag=f"p2-{i}")
                    for i, (c0, cw1) in enumerate(ch1)
                ]

                def l2(j):
                    for i, (c0, cw) in enumerate(chunks):
                        nc.tensor.matmul(
                            p2[i][:, :cw],
                            w2_sb[:, j, ph:ph + 1],
                            h[:, j, c0:c0 + cw],
                            start=(j == 0),
                            stop=(j == 7),
                        )

                for m in range(8):
                    for i, (c0, cw) in enumerate(chunks):
                        cw1 = ch1[i][1]
                        p1 = ps1_pool.tile([P, cw1], F32, name=f"p1_{i}", tag=f"p1-{i}")
                        for k in range(8):
                            nc.tensor.matmul(
                                p1[:, :cw],
                                w1_sb[:, ph, m, k],
                                embT[:, k, c0:c0 + cw],
                                start=(k == 0),
                                stop=(k == 7),
                            )
                        nc.scalar.activation(
                            h[:, m, c0:c0 + cw],
                            p1[:, :cw],
                            mybir.ActivationFunctionType.Relu,
                            bias=b1_sb[:, m, ph:ph + 1],
                        )
                    # deferred second-layer matmul: keeps PE busy with
                    # L1(m) while the activation for m-1 completes
                    if m > 0:
                        l2(m - 1)
                l2(7)

                osb = o_sb[ph]
                for i, (c0, cw) in enumerate(chunks):
                    nc.vector.tensor_scalar_add(
                        osb[:, c0:c0 + cw], p2[i][:, :cw], b2_sb[:, ph:ph + 1]
                    )
                nc.sync.dma_start(outs[ph][:, :], osb)
    nc.finalize()
    return nc


def _route_host(emb, rw, rb):
    logits = emb.astype(np.float32) @ rw.astype(np.float32) + rb.astype(np.float32)
    i1 = np.argmax(logits, axis=1)
    l2m = logits.copy()
    l2m[np.arange(B), i1] = -np.inf
    i2 = np.argmax(l2m, axis=1)
    l1 = logits[np.arange(B), i1]
    l2 = l2m[np.arange(B), i2]
    d = np.exp(l2 - l1)
    wa = (1.0 / (1.0 + d)).astype(np.float32)
    wb = (1.0 - wa).astype(np.float32)
    comb = np.zeros((B, E), np.float32)
    comb[np.arange(B), i1] = wa
    comb[np.arange(B), i2] = wb
    return comb


def kernel(embeddings, router_w, router_b, w1, b1, w2, b2):
    emb = np.ascontiguousarray(np.asarray(embeddings, dtype=np.float32))
    rw = np.asarray(router_w, np.float32)
    rb = np.asarray(router_b, np.float32)
    w1 = np.asarray(w1, np.float32)
    b1 = np.asarray(b1, np.float32)
    w2 = np.asarray(w2, np.float32)
    b2 = np.asarray(b2, np.float32)

    comb = _route_host(emb, rw, rb)
    counts = (comb > 0).sum(axis=0)

    # count-aware schedule: 8 largest experts are phase 0 (one per core),
    # 8 smallest are phase 1; phase length = max count in the phase group.
    ranks = np.argsort(-counts, kind="stable")
    C1 = max(int(counts[ranks[0]]), 1)
    C2 = max(int(counts[ranks[8]]), 1)

    if (C1, C2) not in _CACHE:
        _CACHE[(C1, C2)] = _build(C1, C2)
    nc = _CACHE[(C1, C2)]

    embbf = emb.astype(BF_NP)

    in_maps = []
    toks = []  # per core, per phase: token ids
    for c in range(NCORES):
        es = [int(ranks[c]), int(ranks[8 + c])]
        ctoks = []
        ge_arrs = []
        for ph, e in enumerate(es):
            C = (C1, C2)[ph]
            ids = np.nonzero(comb[:, e] > 0)[0]
            ctoks.append(ids)
            g = np.zeros((C, H), BF_NP)
            g[: len(ids)] = embbf[ids]
            # [C, 8, 128] -> [128(p), 8(hb), C]
            ge_arrs.append(
                np.ascontiguousarray(g.reshape(C, 8, P).transpose(2, 1, 0))
            )
        toks.append(ctoks)
        w1c = np.ascontiguousarray(
            w1[es].reshape(NPH, 8, P, 8, P).transpose(0, 3, 2, 1, 4).astype(BF_NP)
        )
        b1c = np.ascontiguousarray(b1[es].reshape(NPH, 8, P).transpose(2, 1, 0))
        w2c = np.ascontiguousarray(
            w2[es, :, 0].reshape(NPH, 8, P).transpose(2, 1, 0).astype(BF_NP)
        )
        b2c = np.ascontiguousarray(b2[es, 0].reshape(1, NPH))
        in_maps.append({
            "ge0": ge_arrs[0],
            "ge1": ge_arrs[1],
            "w1s": w1c,
            "b1s": b1c,
            "w2s": w2c,
            "b2s": b2c,
        })

    res = run_bass_kernel_spmd(nc, in_maps, core_ids=list(range(NCORES)))

    out = np.zeros((B,), np.float32)
    for c in range(NCORES):
        for ph, e in enumerate([int(ranks[c]), int(ranks[8 + c])]):
            ids = toks[c][ph]
            o = res.results[c][f"out{ph}"][0]
            out[ids] += comb[ids, e] * o[: len(ids)]
    return out.reshape(B, 1)
